# revision 18
# baseline (speedup 1.0000x reference)
"""Trainium2 Bass kernel for additive (Bahdanau-style) attention.

Computation (per batch b):
    mc = context @ wc_w.T + wc_b            # (C, H)
    mq = query  @ wq_w.T + wq_b             # (Q, H)
    emission[q, c] = sum_h we[h] * tanh(mc[c, h] + mq[q, h])   (+ we_b, dropped:
                     softmax is shift-invariant)
    attn = softmax(emission + mask_bias, axis=c)
    wc   = attn @ context                   # (Q, CD)
    out  = tanh(concat([wc, query]) @ lo_w.T + lo_b)

Sharding: data-parallel over (batch, query-half) -> 8 cores, each handling
one batch element and 128 query rows.  No collectives needed.

Per-core kernel strategy:
  - mc_T [h=128p, c=1024f] in SBUF; mq_T [h=128p, q=128f].
  - Main loop over q: one ScalarE tanh over [128, 1024] with per-partition
    bias mq_T[:, q] (broadcast add fused into ACT), then 8 PE matmuls with
    the tanh-tile chunk as stationary and we [128,1] moving, writing
    emission_T column q into per-chunk PSUM tiles (4 chunks per bank).
  - Drain emission_T to SBUF, PE-transpose back to [q, c], one batched
    softmax (reduce_max + ACT exp with accum_out), transposes, and two
    small accumulating matmul chains for attn @ context and the output
    projection.
"""

import sys

_REPO = "/opt/trn_rl_repo"
if _REPO not in sys.path:
    sys.path.insert(0, _REPO)

from contextlib import ExitStack

import ml_dtypes
import numpy as np

import concourse.bass as bass
import concourse.tile as tile
from concourse import mybir
from concourse._compat import with_exitstack
from concourse.bass import ts
from concourse.bass_utils import run_bass_kernel_spmd
from concourse.masks import make_identity

B, Q, C, QD, CD, H = 4, 256, 1024, 512, 512, 128
QL = Q // 2  # query rows per core
N_CORES = 8
F32 = mybir.dt.float32
F32R = mybir.dt.float32r
BF16 = mybir.dt.bfloat16
AF = mybir.ActivationFunctionType
AX = mybir.AxisListType

# bf16 tanh-tile + we for the emission matmuls: 4x faster PE (1 cycle/row vs
# 4 for fp32, plus FWL weight loads).  Measured end-to-end error ~0.4% on attn.
EM_BF16 = True
EDT = BF16 if EM_BF16 else F32


def _emit(ctx: ExitStack, tc: "tile.TileContext", io: dict, pools):
    nc = tc.nc
    singles, tpool, ptrans, psingle, ptail = pools

    ident = singles.tile([128, 128], F32)
    make_identity(nc, ident[:])

    # PE matmuls (incl. transposes) lower to LDWEIGHTS, which this toolchain
    # limits to a single sync-wait.  Tile emits one wait per semaphore the
    # engine hasn't observed yet, so before each phase we issue a throwaway
    # 1-wait PE matmul reading the freshly produced tile; real matmuls that
    # follow then need at most one wait each.  (_split_excess_waits is the
    # backstop for everything else.)
    pescr = psingle.tile([128, 128], F32)

    def pe_touch(ap):
        # ap must be a 2-D slice [K, M] with M <= 128; emits out = ap.T @ ap.
        m = ap.shape[-1]
        nc.tensor.matmul(out=pescr[0:m, 0:m], lhsT=ap, rhs=ap, start=True, stop=True)

    # ---- input loads -------------------------------------------------------
    # context [c, d] as 8 chunks of [c128, d512]; 4 DMAs to spread queues
    ctx_sb = singles.tile([128, 8, CD], F32)
    ctx_r = io["context"].rearrange("(n p) d -> p n d", p=128)
    for i in range(4):
        nc.sync.dma_start(out=ctx_sb[:, 2 * i:2 * i + 2, :], in_=ctx_r[:, 2 * i:2 * i + 2, :])
    q_sb = singles.tile([128, QD], F32)
    nc.sync.dma_start(out=q_sb[:], in_=io["query"])
    wq_sb = singles.tile([H, QD], F32)
    nc.sync.dma_start(out=wq_sb[:], in_=io["wq_w"])
    wc_sb = singles.tile([H, CD], F32)
    nc.sync.dma_start(out=wc_sb[:], in_=io["wc_w"])
    we_sb = singles.tile([H, 1], EDT)
    nc.sync.dma_start(out=we_sb[:], in_=io["we_col"])
    wqb_sb = singles.tile([H, 1], F32)
    nc.sync.dma_start(out=wqb_sb[:], in_=io["wq_b"])
    wcb_sb = singles.tile([H, 1], F32)
    nc.sync.dma_start(out=wcb_sb[:], in_=io["wc_b"])

    def transpose_group(srcs, dst_ap):
        # <=4 PE transposes of [128,128] into one PSUM slot, one DVE copy out.
        p = ptrans.tile([128, 512], F32, tag="pt")
        for j, s in enumerate(srcs):
            nc.tensor.transpose(out=p[:, ts(j, 128)], in_=s, identity=ident[:])
        nc.vector.tensor_copy(out=dst_ap, in_=p[:, 0:128 * len(srcs)])

    # ---- transposes for the pre-matmuls ------------------------------------
    pe_touch(ident[:, 0:128])
    pe_touch(ctx_sb[:, 0, 0:128])
    # context_T [d, c] as [128p, 4dc, 1024c]
    ctxT_sb = singles.tile([128, 4, C], F32)
    for dc in range(4):
        for g in range(2):
            transpose_group([ctx_sb[:, 4 * g + j, ts(dc, 128)] for j in range(4)],
                            ctxT_sb[:, dc, ts(g, 512)])
    # wq_w_T, wc_w_T [d, h] flat [128, 512], dc chunk at ts(dc, 128)
    wqT_sb = singles.tile([128, 512], F32)
    wcT_sb = singles.tile([128, 512], F32)
    pe_touch(wq_sb[:, 0:128])
    pe_touch(wc_sb[:, 0:128])
    transpose_group([wq_sb[:, ts(dc, 128)] for dc in range(4)], wqT_sb[:])
    transpose_group([wc_sb[:, ts(dc, 128)] for dc in range(4)], wcT_sb[:])
    # combined_T [k=1024, q] flat [128, 1024]: k-chunks 0..3 = (attn@context).T
    # (filled later), 4..7 = query_T (now)
    combT_sb = singles.tile([128, 8 * QL], F32)
    pe_touch(q_sb[:, 0:128])
    transpose_group([q_sb[:, ts(dc, 128)] for dc in range(4)], combT_sb[:, 512:1024])

    # ---- mc_T [h, c] and mq_T [h, q] ---------------------------------------
    dvescr = singles.tile([128, 1], F32)
    nc.vector.tensor_copy(out=dvescr[0:H, 0:1], in_=wcb_sb[:])
    nc.vector.tensor_copy(out=dvescr[0:H, 0:1], in_=wqb_sb[:])
    mc_sb = singles.tile([H, C], F32)
    for half in range(2):
        p = ptrans.tile([128, 512], F32, tag="pt")
        for dc in range(4):
            nc.tensor.matmul(out=p[:], lhsT=wcT_sb[:, ts(dc, 128)],
                             rhs=ctxT_sb[:, dc, ts(half, 512)],
                             start=(dc == 0), stop=(dc == 3))
        nc.vector.tensor_scalar_add(out=mc_sb[:, ts(half, 512)], in0=p[:], scalar1=wcb_sb[:, 0:1])
    mq_sb = singles.tile([H, QL], F32)
    pq = ptrans.tile([128, 512], F32, tag="pt")
    for dc in range(4):
        nc.tensor.matmul(out=pq[:, 0:QL], lhsT=wqT_sb[:, ts(dc, 128)],
                         rhs=combT_sb[:, ts(4 + dc, 128)],
                         start=(dc == 0), stop=(dc == 3))
    nc.vector.tensor_scalar_add(out=mq_sb[:], in0=pq[:, 0:QL], scalar1=wqb_sb[:, 0:1])

    # ---- main loop: emission computed transposed, column q per iteration ---
    # emission_T[c, q] = sum_h t[h, c] * we[h]; lhsT = tanh-tile chunk
    # (stationary [128h, 128c]), rhs = we [128h, 1].  Four [c128, q128] chunks
    # packed per PSUM bank.
    emT_lo = psingle.tile([128, 4, QL], F32)
    emT_hi = psingle.tile([128, 4, QL], F32)
    pe_touch(we_sb[:, 0:1])
    for q in range(QL):
        t = tpool.tile([H, C], EDT, tag="t_tanh")
        nc.scalar.activation(out=t[:], in_=mc_sb[:], func=AF.Tanh,
                             bias=mq_sb[:, q:q + 1], scale=1.0)
        for cc in range(8):
            dst = emT_lo if cc < 4 else emT_hi
            nc.tensor.matmul(out=dst[:, cc % 4, q:q + 1], lhsT=t[:, ts(cc, 128)],
                             rhs=we_sb[:], start=True, stop=True)

    # ---- deferred loads/transposes (only needed after the main loop; the
    # scheduler overlaps them with it) --------------------------------------
    lob_sb = singles.tile([128, 4], F32)
    nc.sync.dma_start(out=lob_sb[:], in_=io["lo_b2d"])
    actscr = singles.tile([128, 1], F32)
    nc.scalar.copy(out=actscr[:], in_=lob_sb[:, 0:1])
    # lo_w [o, k] as 4 chunks of [o128, k1024]
    low_sb = singles.tile([128, 4, QD + CD], F32)
    low_r = io["lo_w"].rearrange("(n p) k -> p n k", p=128)
    for i in range(2):
        nc.sync.dma_start(out=low_sb[:, 2 * i:2 * i + 2, :], in_=low_r[:, 2 * i:2 * i + 2, :])
    mb_sb = singles.tile([1, C], F32)
    nc.sync.dma_start(out=mb_sb[:], in_=io["mask_bias"])
    # lo_w_T [k, o] as [128p, 8kc, 512o]
    lowT_sb = singles.tile([128, 8, QD], F32)
    pe_touch(low_sb[:, 0, 0:128])
    for kc in range(8):
        transpose_group([low_sb[:, oc, ts(kc, 128)] for oc in range(4)],
                        lowT_sb[:, kc, :])
    # mask bias broadcast to [128, 1024]
    ones_sb = singles.tile([1, 128], F32)
    nc.vector.memset(ones_sb[:], 1.0)
    maskbc_sb = singles.tile([128, C], F32)
    pe_touch(mb_sb[0:1, 0:128])
    for half in range(2):
        p = ptrans.tile([128, 512], F32, tag="pt")
        nc.tensor.matmul(out=p[:], lhsT=ones_sb[0:1, :], rhs=mb_sb[0:1, ts(half, 512)],
                         start=True, stop=True)
        nc.vector.tensor_copy(out=maskbc_sb[:, ts(half, 512)], in_=p[:])

    # ---- drain emission_T to SBUF, transpose back to [q, c] ----------------
    emT_sb = singles.tile([128, C], F32)
    nc.vector.tensor_copy(out=emT_sb[:, 0:512], in_=emT_lo[:])
    nc.vector.tensor_copy(out=emT_sb[:, 512:1024], in_=emT_hi[:])
    em_sb = singles.tile([QL, C], F32)
    for g in range(2):
        transpose_group([emT_sb[:, ts(4 * g + j, 128)] for j in range(4)],
                        em_sb[:, ts(g, 512)])

    # ---- softmax over c (free dim) -----------------------------------------
    em2_sb = singles.tile([QL, C], F32)
    nc.vector.tensor_add(out=em2_sb[:], in0=em_sb[:], in1=maskbc_sb[:])
    mx_sb = singles.tile([QL, 1], F32)
    nc.vector.reduce_max(out=mx_sb[:], in_=em2_sb[:], axis=AX.X)
    negmx_sb = singles.tile([QL, 1], F32)
    nc.vector.tensor_scalar_mul(out=negmx_sb[:], in0=mx_sb[:], scalar1=-1.0)
    expsum_sb = singles.tile([QL, 1], F32)
    attnu_sb = singles.tile([QL, C], F32)
    nc.scalar.activation(out=attnu_sb[:], in_=em2_sb[:], func=AF.Exp,
                         bias=negmx_sb[:, 0:1], scale=1.0, accum_out=expsum_sb[:, 0:1])
    rec_sb = singles.tile([QL, 1], F32)
    nc.vector.reciprocal(out=rec_sb[:], in_=expsum_sb[:])
    attn_sb = singles.tile([QL, C], F32)
    nc.vector.tensor_scalar_mul(out=attn_sb[:], in0=attnu_sb[:], scalar1=rec_sb[:, 0:1])
    nc.sync.dma_start(out=io["attn"], in_=attn_sb[:])

    # ---- attn_T [c, q] flat [128, 1024], cc chunk at ts(cc, 128) -----------
    attnT_sb = singles.tile([128, 8 * QL], F32)
    for g in range(2):
        transpose_group([attn_sb[:, ts(4 * g + j, 128)] for j in range(4)],
                        attnT_sb[:, ts(g, 512)])

    # ---- weighted context: (attn @ context).T = ctx-chunks.T @ attn_T ------
    for dc in range(4):
        acc = ptail.tile([128, QL], F32, tag="acc")
        for cc in range(8):
            nc.tensor.matmul(out=acc[:], lhsT=ctx_sb[:, cc, ts(dc, 128)],
                             rhs=attnT_sb[:, ts(cc, 128)],
                             start=(cc == 0), stop=(cc == 7))
        nc.vector.tensor_copy(out=combT_sb[:, ts(dc, 128)], in_=acc[:])

    # ---- output projection: out_T[o, q] = lo_w_T.T @ comb_T, + bias, tanh --
    outT_sb = singles.tile([128, 4, QL], F32)
    for oc in range(4):
        acc = ptail.tile([128, QL], F32, tag="acc")
        for kc in range(8):
            nc.tensor.matmul(out=acc[:], lhsT=lowT_sb[:, kc, ts(oc, 128)],
                             rhs=combT_sb[:, ts(kc, 128)],
                             start=(kc == 0), stop=(kc == 7))
        nc.scalar.activation(out=outT_sb[:, oc, :], in_=acc[:], func=AF.Tanh,
                             bias=lob_sb[:, oc:oc + 1], scale=1.0)

    # ---- transpose back to [q, o] and store --------------------------------
    out_sb = singles.tile([QL, QD], F32)
    for oc in range(4):
        pe_touch(outT_sb[:, oc, 0:128])
    transpose_group([outT_sb[:, oc, :] for oc in range(4)], out_sb[:])
    nc.sync.dma_start(out=io["output"], in_=out_sb[:])


@with_exitstack
def _body(ctx: ExitStack, tc: "tile.TileContext", io: dict, reps: int = 1,
          loop_n: int = 1):
    nc = tc.nc
    singles = ctx.enter_context(tc.tile_pool(name="singles", bufs=1))
    tpool = ctx.enter_context(tc.tile_pool(name="tanh", bufs=4))
    ptrans = ctx.enter_context(tc.tile_pool(name="ptrans", bufs=2, space="PSUM"))
    psingle = ctx.enter_context(tc.tile_pool(name="psingle", bufs=1, space="PSUM"))
    ptail = ctx.enter_context(tc.tile_pool(name="ptail", bufs=2, space="PSUM"))
    pools = (singles, tpool, ptrans, psingle, ptail)
    if loop_n > 1:
        hints = (mybir.EngineType.PE, mybir.EngineType.Activation,
                 mybir.EngineType.DVE, mybir.EngineType.SP)
        with tc.For_i(0, loop_n, 1, hint_engines=hints):
            _emit(ctx, tc, io, pools)
    else:
        for _ in range(reps):
            _emit(ctx, tc, io, pools)


def _split_excess_waits(nc):
    """This toolchain's walrus accepts only ONE sync-wait per instruction.
    Peel extra waits off into standalone EventSemaphore ops on the same
    engine immediately before the instruction (engine streams are in-order,
    so semantics are preserved)."""
    skip = (mybir.InstEventSemaphore,)
    for bb in nc.m.functions[0].blocks:
        insts = bb.instructions
        new = []
        changed = False
        for inst in insts:
            si = inst.sync_info
            if (si is not None and si.on_wait and len(si.on_wait) > 1
                    and not isinstance(inst, skip)):
                waits = list(si.on_wait)
                for k, w in enumerate(waits[:-1]):
                    ev = mybir.InstEventSemaphore(name=f"{inst.name}-evw{k}")
                    ev.engine = inst.engine
                    ev.sync_info = mybir.SyncInfo(on_wait=[w], on_update=[])
                    nc.inst_map[ev.name] = ev
                    new.append(ev)
                inst.sync_info = mybir.SyncInfo(
                    on_wait=[waits[-1]], on_update=list(si.on_update or []))
                changed = True
            new.append(inst)
        if changed:
            bb.instructions = new


def build_program(reps: int = 1, loop_n: int = 1):
    nc = bass.Bass("TRN2", target_bir_lowering=False, debug=False, num_devices=N_CORES)

    def din(name, shape, dt=F32):
        return nc.dram_tensor(name, shape, dt, kind="ExternalInput").ap()

    io = {
        "query": din("query", [QL, QD]),
        "context": din("context", [C, CD]),
        "mask_bias": din("mask_bias", [1, C]),
        "wq_w": din("wq_w", [H, QD]),
        "wq_b": din("wq_b", [H, 1]),
        "wc_w": din("wc_w", [H, CD]),
        "wc_b": din("wc_b", [H, 1]),
        "we_col": din("we_col", [H, 1], EDT),
        "lo_w": din("lo_w", [QD, QD + CD]),
        "lo_b2d": din("lo_b2d", [128, 4]),
        "output": nc.dram_tensor("output", [QL, QD], F32, kind="ExternalOutput").ap(),
        "attn": nc.dram_tensor("attn", [QL, C], F32, kind="ExternalOutput").ap(),
    }
    with tile.TileContext(nc) as tc:
        _body(tc, io, reps=reps, loop_n=loop_n)
    _split_excess_waits(nc)
    return nc


_CACHE = {}


def _get_program(reps: int = 1):
    key = ("nc", reps)
    if key not in _CACHE:
        _CACHE[key] = build_program(reps)
    return _CACHE[key]


def make_in_maps(query, context, mask, wq_w, wq_b, wc_w, wc_b, we_w, we_b, lo_w, lo_b):
    f = lambda x: np.ascontiguousarray(np.asarray(x, dtype=np.float32))
    we_col = np.asarray(we_w, np.float32).reshape(H, 1)
    if EM_BF16:
        we_col = we_col.astype(ml_dtypes.bfloat16)
    shared = {
        "wq_w": f(wq_w), "wq_b": f(wq_b).reshape(H, 1),
        "wc_w": f(wc_w), "wc_b": f(wc_b).reshape(H, 1),
        "we_col": np.ascontiguousarray(we_col),
        "lo_w": f(lo_w), "lo_b2d": f(np.asarray(lo_b, np.float32).reshape(4, 128).T),
    }
    mask = np.asarray(mask, bool)
    in_maps = []
    for core in range(N_CORES):
        b, h2 = divmod(core, 2)
        q0 = h2 * QL
        in_maps.append(dict(
            query=f(np.asarray(query)[b, q0:q0 + QL]),
            context=f(np.asarray(context)[b]),
            mask_bias=np.where(mask[b], 0.0, -1e30).astype(np.float32).reshape(1, C),
            **shared,
        ))
    return in_maps


def kernel(query, context, mask, wq_w, wq_b, wc_w, wc_b, we_w, we_b, lo_w, lo_b,
           _trace=False):
    nc = _get_program()
    in_maps = make_in_maps(query, context, mask, wq_w, wq_b, wc_w, wc_b, we_w, we_b,
                           lo_w, lo_b)
    res = run_bass_kernel_spmd(nc, in_maps, list(range(N_CORES)), trace=_trace)
    output = np.empty((B, Q, QD), np.float32)
    attn = np.empty((B, Q, C), np.float32)
    for core in range(N_CORES):
        b, h2 = divmod(core, 2)
        q0 = h2 * QL
        output[b, q0:q0 + QL] = res.results[core]["output"]
        attn[b, q0:q0 + QL] = res.results[core]["attn"]
    if _trace:
        return (output, attn), res
    return output, attn


# revision 27
# speedup vs baseline: 1.1814x; 1.1814x over previous
"""Trainium2 Bass kernel for additive (Bahdanau-style) attention.

Computation (per batch b):
    mc = context @ wc_w.T + wc_b            # (C, H)
    mq = query  @ wq_w.T + wq_b             # (Q, H)
    emission[q, c] = sum_h we[h] * tanh(mc[c, h] + mq[q, h])   (+ we_b, dropped:
                     softmax is shift-invariant)
    attn = softmax(emission + mask_bias, axis=c)
    wc   = attn @ context                   # (Q, CD)
    out  = tanh(concat([wc, query]) @ lo_w.T + lo_b)

Sharding: data-parallel over (batch, query-half) -> 8 cores, each handling
one batch element and 128 query rows.  No collectives needed.

Per-core kernel strategy:
  - mc_T [h=128p, c=1024f] in SBUF; mq_T [h=128p, q=128f].
  - Main loop over q: one ScalarE tanh over [128, 1024] with per-partition
    bias mq_T[:, q] (broadcast add fused into ACT), then 8 PE matmuls with
    the tanh-tile chunk as stationary and we [128,1] moving, writing
    emission_T column q into per-chunk PSUM tiles (4 chunks per bank).
  - Drain emission_T to SBUF, PE-transpose back to [q, c], one batched
    softmax (reduce_max + ACT exp with accum_out), transposes, and two
    small accumulating matmul chains for attn @ context and the output
    projection.
"""

import sys

_REPO = "/opt/trn_rl_repo"
if _REPO not in sys.path:
    sys.path.insert(0, _REPO)

from contextlib import ExitStack

import ml_dtypes
import numpy as np

import concourse.bass as bass
import concourse.tile as tile
from concourse import mybir
from concourse._compat import with_exitstack
from concourse.bass import ts
from concourse.bass_utils import run_bass_kernel_spmd
from concourse.masks import make_identity

B, Q, C, QD, CD, H = 4, 256, 1024, 512, 512, 128
QL = Q // 2  # query rows per core
N_CORES = 8
F32 = mybir.dt.float32
F32R = mybir.dt.float32r
BF16 = mybir.dt.bfloat16
AF = mybir.ActivationFunctionType
AX = mybir.AxisListType

# bf16 tanh-tile + we for the emission matmuls: 4x faster PE (1 cycle/row vs
# 4 for fp32, plus FWL weight loads).  Measured end-to-end error ~0.4% on attn.
EM_BF16 = True
EDT = BF16 if EM_BF16 else F32


def _emit(ctx: ExitStack, tc: "tile.TileContext", io: dict, pools):
    nc = tc.nc
    singles, tpool, spool, ptrans, psingle, ptail = pools[:6]

    ident = singles.tile([128, 128], F32)
    make_identity(nc, ident[:])

    # PE matmuls (incl. transposes) lower to LDWEIGHTS, which this toolchain
    # limits to a single sync-wait.  Tile emits one wait per semaphore the
    # engine hasn't observed yet, so before each phase we issue a throwaway
    # 1-wait PE matmul reading the freshly produced tile; real matmuls that
    # follow then need at most one wait each.  (_split_excess_waits is the
    # backstop for everything else.)
    pescr = ptail.tile([QL, QD], F32, tag="acc")

    def pe_touch(ap):
        # ap must be a 2-D slice [K, M] with M <= 128; emits out = ap.T @ ap.
        m = ap.shape[-1]
        nc.tensor.matmul(out=pescr[0:m, 0:m], lhsT=ap, rhs=ap, start=True, stop=True)

    # ---- input loads: small tensors first so they are not queued behind the
    # 2 MB context transfer (wc_w gates the mc chain) ------------------------
    wc_sb = singles.tile([H, CD], F32)
    nc.sync.dma_start(out=wc_sb[:], in_=io["wc_w"])
    wq_sb = singles.tile([H, QD], F32)
    nc.sync.dma_start(out=wq_sb[:], in_=io["wq_w"])
    q_sb = singles.tile([128, QD], F32)
    nc.sync.dma_start(out=q_sb[:], in_=io["query"])
    we_sb = singles.tile([H, 1], EDT)
    nc.sync.dma_start(out=we_sb[:], in_=io["we_col"])
    wqb_sb = singles.tile([H, 1], F32)
    nc.sync.dma_start(out=wqb_sb[:], in_=io["wq_b"])
    wcb_sb = singles.tile([H, 1], F32)
    nc.sync.dma_start(out=wcb_sb[:], in_=io["wc_b"])
    maskT_sb = singles.tile([128, 8], F32)
    nc.sync.dma_start(out=maskT_sb[:], in_=io["maskT"])
    lob_sb = singles.tile([1, QD], F32)
    nc.sync.dma_start(out=lob_sb[:], in_=io["lo_brow"])
    # context [c, d] as 8 chunks of [c128, d512]; 8 DMAs to spread queues
    ctx_sb = singles.tile([128, 8, CD], F32)
    ctx_r = io["context"].rearrange("(n p) d -> p n d", p=128)
    for i in range(8):
        nc.sync.dma_start(out=ctx_sb[:, i:i + 1, :], in_=ctx_r[:, i:i + 1, :])

    def transpose_group(srcs, dst_ap, eng=None):
        # <=4 PE transposes of [128,128] into one PSUM slot, one copy out
        # (DVE by default; ACT when it's otherwise idle).
        p = ptrans.tile([128, 512], F32, tag="pt")
        for j, s in enumerate(srcs):
            nc.tensor.transpose(out=p[:, ts(j, 128)], in_=s, identity=ident[:])
        if eng == "act":
            nc.scalar.copy(out=dst_ap, in_=p[:, 0:128 * len(srcs)])
        else:
            nc.vector.tensor_copy(out=dst_ap, in_=p[:, 0:128 * len(srcs)])

    # ---- transposes for the pre-matmuls, in data-arrival order -------------
    pe_touch(ident[:, 0:128])
    # wq_w_T, wc_w_T [d, h] flat [128, 512], dc chunk at ts(dc, 128)
    wqT_sb = singles.tile([128, 512], F32R)
    wcT_sb = singles.tile([128, 512], F32R)
    pe_touch(wc_sb[:, 0:128])
    pe_touch(wq_sb[:, 0:128])
    transpose_group([wc_sb[:, ts(dc, 128)] for dc in range(4)], wcT_sb[:])
    transpose_group([wq_sb[:, ts(dc, 128)] for dc in range(4)], wqT_sb[:])
    # combined_T [k=1024, q] flat [128, 1024] float32r: k-chunks 0..3 =
    # (attn@context).T (filled in the tail), 4..7 = query_T (now)
    combT_sb = singles.tile([128, 8 * QL], F32R)
    pe_touch(q_sb[:, 0:128])
    transpose_group([q_sb[:, ts(dc, 128)] for dc in range(4)], combT_sb[:, 512:1024])
    # context_T [d, c] as [128p, 4dc, 1024c] — float32r (rounded by the DVE
    # copy) so the mc matmuls run at 1 cycle/row instead of fp32's 4
    pe_touch(ctx_sb[:, 0, 0:128])
    ctxT_sb = singles.tile([128, 4, C], F32R)
    for g in range(2):
        for dc in range(4):
            transpose_group([ctx_sb[:, 4 * g + j, ts(dc, 128)] for j in range(4)],
                            ctxT_sb[:, dc, ts(g, 512)], eng="act" if g else None)

    # ---- mq' [h, q] with BOTH biases folded (mq + wq_b + wc_b), and the mask
    # bias / lo_b broadcasts ------------------------------------------------
    dvescr = singles.tile([128, 1], F32)
    nc.vector.tensor_copy(out=dvescr[0:H, 0:1], in_=wcb_sb[:])
    nc.vector.tensor_copy(out=dvescr[0:H, 0:1], in_=wqb_sb[:])
    mq_sb = singles.tile([H, QL], F32)
    pq = ptrans.tile([128, 512], F32, tag="pt")
    for dc in range(4):
        nc.tensor.matmul(out=pq[:, 0:QL], lhsT=wqT_sb[:, ts(dc, 128)],
                         rhs=combT_sb[:, ts(4 + dc, 128)],
                         start=(dc == 0), stop=(dc == 3))
    nc.vector.tensor_scalar_add(out=mq_sb[:], in0=pq[:, 0:QL], scalar1=wqb_sb[:, 0:1])
    nc.vector.tensor_scalar_add(out=mq_sb[:], in0=mq_sb[:], scalar1=wcb_sb[:, 0:1])
    # ---- mc_T [h, c] raw (biases live in mq'), bf16 for the DVE adds -------
    mc_sb = singles.tile([H, C], BF16)
    for half in range(2):
        p = ptrans.tile([128, 512], F32, tag="pt")
        for dc in range(4):
            nc.tensor.matmul(out=p[:], lhsT=wcT_sb[:, ts(dc, 128)],
                             rhs=ctxT_sb[:, dc, ts(half, 512)],
                             start=(dc == 0), stop=(dc == 3))
        if half:
            nc.scalar.copy(out=mc_sb[:, ts(half, 512)], in_=p[:])
        else:
            nc.vector.tensor_copy(out=mc_sb[:, ts(half, 512)], in_=p[:])

    # ---- main loop: emission computed transposed, column q per iteration ---
    # emission_T[c, q] = sum_h t[h, c] * we[h]; lhsT = tanh-tile chunk
    # (stationary [128h, 128c]), rhs = we [128h, 1].  Four [c128, q128] chunks
    # packed per PSUM bank.
    emT_lo = psingle.tile([128, 4, QL], F32)
    emT_hi = psingle.tile([128, 4, QL], F32)
    pe_touch(we_sb[:, 0:1])
    QB = 8  # q rows per block
    for blk in range(QL // QB):
        s = spool.tile([H, QB, C], BF16, tag="s_add")
        for j in range(QB):
            q = blk * QB + j
            nc.vector.tensor_scalar_add(out=s[:, j, :], in0=mc_sb[:],
                                        scalar1=mq_sb[:, q:q + 1])
        t = tpool.tile([H, QB, C], BF16, tag="t_tanh")
        nc.scalar.activation(out=t[:], in_=s[:], func=AF.Tanh)
        for j in range(QB):
            q = blk * QB + j
            for cc in range(8):
                dst = emT_lo if cc < 4 else emT_hi
                nc.tensor.matmul(out=dst[:, cc % 4, q:q + 1],
                                 lhsT=t[:, j, ts(cc, 128)],
                                 rhs=we_sb[:], start=True, stop=True)

    nc.vector.tensor_copy(out=dvescr[0:128, 0:1], in_=maskT_sb[:, 0:1])

    # ---- output-projection PSUM: pre-accumulate lo_b and the query half of
    # combined during the main loop; the wc half lands later -----------------
    ones_sb = singles.tile([1, 128], F32)
    nc.vector.memset(ones_sb[:], 1.0)
    pe_touch(lob_sb[0:1, 0:128])
    out_ps = ptail.tile([QL, QD], F32, tag="acc")
    nc.tensor.matmul(out=out_ps[:], lhsT=ones_sb[0:1, :], rhs=lob_sb[0:1, :],
                     start=True, stop=False, skip_group_check=True)

    # ---- float32r recasts for the tail, overlapped with the main loop ------
    ctxr_sb = singles.tile([128, 8, CD], F32R)
    nc.vector.tensor_copy(out=ctxr_sb[:], in_=ctx_sb[:])
    lowT_sb = singles.tile([128, 8, QD], F32)
    lowT_r = io["lo_wT"].rearrange("(n p) o -> p n o", p=128)
    for i in range(2):
        nc.sync.dma_start(out=lowT_sb[:, 4 * i:4 * i + 4, :], in_=lowT_r[:, 4 * i:4 * i + 4, :])
    lowTr_sb = singles.tile([128, 8, QD], F32R)
    nc.vector.tensor_copy(out=lowTr_sb[:], in_=lowT_sb[:])
    for kc in range(4, 8):
        nc.tensor.matmul(out=out_ps[:], lhsT=combT_sb[:, ts(kc, 128)],
                         rhs=lowTr_sb[:, kc, :], start=False, stop=False,
                         skip_group_check=True)

    # ---- drain emission_T to SBUF with the mask folded in (per-partition
    # scalar in the [c, q] layout), split across DVE and ACT -----------------
    emT_sb = singles.tile([128, C], F32)
    for cc in range(8):
        srcp = emT_lo if cc < 4 else emT_hi
        if cc % 2:
            nc.scalar.activation(out=emT_sb[:, ts(cc, 128)], in_=srcp[:, cc % 4, :],
                                 func=AF.Identity, bias=maskT_sb[:, cc:cc + 1], scale=1.0)
        else:
            nc.vector.tensor_scalar_add(out=emT_sb[:, ts(cc, 128)], in0=srcp[:, cc % 4, :],
                                        scalar1=maskT_sb[:, cc:cc + 1])
    em_sb = singles.tile([QL, C], F32)
    for g in range(2):
        transpose_group([emT_sb[:, ts(4 * g + j, 128)] for j in range(4)],
                        em_sb[:, ts(g, 512)], eng="act" if g else None)

    # ---- softmax over c (free dim); normalization rides on the wc drain ----
    mx_sb = singles.tile([QL, 1], F32)
    nc.vector.reduce_max(out=mx_sb[:], in_=em_sb[:], axis=AX.X)
    negmx_sb = singles.tile([QL, 1], F32)
    nc.vector.tensor_scalar_mul(out=negmx_sb[:], in0=mx_sb[:], scalar1=-1.0)
    expsum_sb = singles.tile([QL, 1], F32)
    attnu_sb = singles.tile([QL, C], F32)
    nc.scalar.activation(out=attnu_sb[:], in_=em_sb[:], func=AF.Exp,
                         bias=negmx_sb[:, 0:1], scale=1.0, accum_out=expsum_sb[:, 0:1])
    rec_sb = singles.tile([QL, 1], F32)
    nc.vector.reciprocal(out=rec_sb[:], in_=expsum_sb[:])
    attn_sb = singles.tile([QL, C], F32)
    nc.vector.tensor_scalar_mul(out=attn_sb[:], in0=attnu_sb[:], scalar1=rec_sb[:, 0:1])
    nc.sync.dma_start(out=io["attn"], in_=attn_sb[:])

    # ---- attn_T (unnormalized) [c, q] flat [128, 1024] float32r ------------
    attnT_sb = singles.tile([128, 8 * QL], F32R)
    for g in range(2):
        transpose_group([attnu_sb[:, ts(4 * g + j, 128)] for j in range(4)],
                        attnT_sb[:, ts(g, 512)], eng="act" if g else None)

    # ---- weighted context (unnormalized), natural [q, d], N=512 float32r ---
    wc_ps = ptail.tile([QL, CD], F32, tag="acc")
    for cc in range(8):
        nc.tensor.matmul(out=wc_ps[:], lhsT=attnT_sb[:, ts(cc, 128)],
                         rhs=ctxr_sb[:, cc, :], start=(cc == 0), stop=(cc == 7))
    # normalization folded into the PSUM drain (per-partition 1/sum)
    wcn_sb = singles.tile([QL, CD], F32)
    nc.vector.tensor_scalar_mul(out=wcn_sb[:], in0=wc_ps[:], scalar1=rec_sb[:, 0:1])
    transpose_group([wcn_sb[:, ts(dc, 128)] for dc in range(4)], combT_sb[:, 0:512])

    # ---- output projection: add the wc half into the pre-accumulated PSUM --
    for kc in range(4):
        nc.tensor.matmul(out=out_ps[:], lhsT=combT_sb[:, ts(kc, 128)],
                         rhs=lowTr_sb[:, kc, :], start=False, stop=(kc == 3),
                         skip_group_check=True)
    out_sb = singles.tile([QL, QD], F32)
    nc.scalar.activation(out=out_sb[:], in_=out_ps[:], func=AF.Tanh)
    nc.sync.dma_start(out=io["output"], in_=out_sb[:])


@with_exitstack
def _body(ctx: ExitStack, tc: "tile.TileContext", io: dict, reps: int = 1,
          loop_n: int = 1):
    nc = tc.nc
    singles = ctx.enter_context(tc.tile_pool(name="singles", bufs=1))
    tpool = ctx.enter_context(tc.tile_pool(name="tanh", bufs=2))
    spool = ctx.enter_context(tc.tile_pool(name="sadd", bufs=2))
    ptrans = ctx.enter_context(tc.tile_pool(name="ptrans", bufs=3, space="PSUM"))
    psingle = ctx.enter_context(tc.tile_pool(name="psingle", bufs=1, space="PSUM"))
    ptail = ctx.enter_context(tc.tile_pool(name="ptail", bufs=3, space="PSUM"))
    pools = (singles, tpool, spool, ptrans, psingle, ptail)
    if loop_n > 1:
        hints = (mybir.EngineType.PE, mybir.EngineType.Activation,
                 mybir.EngineType.DVE, mybir.EngineType.SP)
        with tc.For_i(0, loop_n, 1, hint_engines=hints):
            _emit(ctx, tc, io, pools)
    else:
        for _ in range(reps):
            _emit(ctx, tc, io, pools)


def _split_excess_waits(nc):
    """This toolchain's walrus accepts only ONE sync-wait per instruction.
    Peel extra waits off into standalone EventSemaphore ops on the same
    engine immediately before the instruction (engine streams are in-order,
    so semantics are preserved)."""
    skip = (mybir.InstEventSemaphore,)
    for bb in nc.m.functions[0].blocks:
        insts = bb.instructions
        new = []
        changed = False
        for inst in insts:
            si = inst.sync_info
            if (si is not None and si.on_wait and len(si.on_wait) > 1
                    and not isinstance(inst, skip)):
                waits = list(si.on_wait)
                for k, w in enumerate(waits[:-1]):
                    ev = mybir.InstEventSemaphore(name=f"{inst.name}-evw{k}")
                    ev.engine = inst.engine
                    ev.sync_info = mybir.SyncInfo(on_wait=[w], on_update=[])
                    nc.inst_map[ev.name] = ev
                    new.append(ev)
                inst.sync_info = mybir.SyncInfo(
                    on_wait=[waits[-1]], on_update=list(si.on_update or []))
                changed = True
            new.append(inst)
        if changed:
            bb.instructions = new


def build_program(reps: int = 1, loop_n: int = 1):
    nc = bass.Bass("TRN2", target_bir_lowering=False, debug=False, num_devices=N_CORES)

    def din(name, shape, dt=F32):
        return nc.dram_tensor(name, shape, dt, kind="ExternalInput").ap()

    io = {
        "query": din("query", [QL, QD]),
        "context": din("context", [C, CD]),
        "maskT": din("maskT", [128, 8]),
        "wq_w": din("wq_w", [H, QD]),
        "wq_b": din("wq_b", [H, 1]),
        "wc_w": din("wc_w", [H, CD]),
        "wc_b": din("wc_b", [H, 1]),
        "we_col": din("we_col", [H, 1], EDT),
        "lo_wT": din("lo_wT", [QD + CD, QD]),
        "lo_brow": din("lo_brow", [1, QD]),
        "output": nc.dram_tensor("output", [QL, QD], F32, kind="ExternalOutput").ap(),
        "attn": nc.dram_tensor("attn", [QL, C], F32, kind="ExternalOutput").ap(),
    }
    with tile.TileContext(nc) as tc:
        _body(tc, io, reps=reps, loop_n=loop_n)
    _split_excess_waits(nc)
    return nc


_CACHE = {}


def _get_program(reps: int = 1):
    key = ("nc", reps)
    if key not in _CACHE:
        _CACHE[key] = build_program(reps)
    return _CACHE[key]


def make_in_maps(query, context, mask, wq_w, wq_b, wc_w, wc_b, we_w, we_b, lo_w, lo_b):
    f = lambda x: np.ascontiguousarray(np.asarray(x, dtype=np.float32))
    we_col = np.asarray(we_w, np.float32).reshape(H, 1)
    if EM_BF16:
        we_col = we_col.astype(ml_dtypes.bfloat16)
    shared = {
        "wq_w": f(wq_w), "wq_b": f(wq_b).reshape(H, 1),
        "wc_w": f(wc_w), "wc_b": f(wc_b).reshape(H, 1),
        "we_col": np.ascontiguousarray(we_col),
        "lo_wT": f(np.asarray(lo_w, np.float32).T),
        "lo_brow": f(np.asarray(lo_b, np.float32).reshape(1, QD)),
    }
    mask = np.asarray(mask, bool)
    in_maps = []
    for core in range(N_CORES):
        b, h2 = divmod(core, 2)
        q0 = h2 * QL
        in_maps.append(dict(
            query=f(np.asarray(query)[b, q0:q0 + QL]),
            context=f(np.asarray(context)[b]),
            maskT=np.ascontiguousarray(
                np.where(mask[b], 0.0, -1e30).astype(np.float32).reshape(8, 128).T),
            **shared,
        ))
    return in_maps


def kernel(query, context, mask, wq_w, wq_b, wc_w, wc_b, we_w, we_b, lo_w, lo_b,
           _trace=False):
    nc = _get_program()
    in_maps = make_in_maps(query, context, mask, wq_w, wq_b, wc_w, wc_b, we_w, we_b,
                           lo_w, lo_b)
    res = run_bass_kernel_spmd(nc, in_maps, list(range(N_CORES)), trace=_trace)
    output = np.empty((B, Q, QD), np.float32)
    attn = np.empty((B, Q, C), np.float32)
    for core in range(N_CORES):
        b, h2 = divmod(core, 2)
        q0 = h2 * QL
        output[b, q0:q0 + QL] = res.results[core]["output"]
        attn[b, q0:q0 + QL] = res.results[core]["attn"]
    if _trace:
        return (output, attn), res
    return output, attn
